# revision 32
# baseline (speedup 1.0000x reference)
"""Trainium2 Bass kernel for a transformer decoder block (self-attn + cross-attn + MLP).

Sharding: 8 cores = 2 batch groups x 4 cores. Within a group, core c owns
rows r = c (mod 4) of its batch (strided rows balance causal attention work
while keeping the compiled program identical across cores). K/V are computed
replicated within a group (no collectives; cores are fully independent).

v3 (all-bf16): bf16 weights and activations everywhere (halves DMA and
transpose cost vs f32); K/V builds consume transposed-h tiles directly from
a rolling SBUF window (no DRAM staging); key blocks processed in pairs to
halve exp/mask instruction count; cross K/V built from DRAM-streamed yT and
emitted after the self-attention inner loop so the list scheduler fills PE
gaps under the ACT-bound exp phase; h4 staged in SBUF.
"""

import os
import sys

for _p in ("/opt/trn_rl_repo", "/root/.axon_site/_ro/trn_rl_repo"):
    if os.path.isdir(_p) and _p not in sys.path:
        sys.path.insert(0, _p)

import numpy as np
import ml_dtypes

BF16NP = ml_dtypes.bfloat16

B, N, C, H, Y_DIM, HID = 2, 2048, 1024, 16, 1024, 4096
HD = C // H
SCALE = HD ** -0.5
EPS = 1e-5

G = 2           # batch groups
CPG = 4         # cores per group
R = N // CPG    # rows per core (512)
RT = R // 128   # row tiles per core (4)
KB = N // 128   # key blocks (16)
NP = KB // 2    # key block pairs (8)
KIN = C // 128  # contraction tiles for C (8)

_CACHE = {}


# ---------------------------------------------------------------------------
# program builder
# ---------------------------------------------------------------------------

def _build(mode, skip_gb, skip_bias):
    """mode: 'causal' | 'none' | 'dense'"""
    import concourse.bass as bass
    import concourse.mybir as mybir
    import concourse.tile as tile
    from concourse import bacc
    from concourse.masks import make_identity

    dt = mybir.dt
    F32, BF16 = dt.float32, dt.bfloat16
    AF = mybir.ActivationFunctionType
    ALU = mybir.AluOpType

    nc = bacc.Bacc("TRN2", target_bir_lowering=False, debug=False, num_devices=8)

    # ---- DRAM I/O ----------------------------------------------------------
    def din(name, shape, dtype=None):
        return nc.dram_tensor(name, list(shape), dtype or F32,
                              kind="ExternalInput").ap()

    x_my = din("x_my", (R, C))
    x_full = din("x_full", (N, C))
    yTb = din("yTb", (Y_DIM, N), BF16)
    yBlk = din("yBlk", (KB * 128, KIN * 128), BF16)
    wqkT = din("wqkT", (C, 2 * C), BF16)     # [in, out] cols 0:C q, C:2C k
    wvT = din("wvT", (C, C), BF16)
    wq2T = din("wq2T", (C, C), BF16)
    wkv2T = din("wkv2T", (Y_DIM, 2 * C), BF16)  # cols 0:C k, C:2C v
    wprojT = din("wprojT", (C, C), BF16)
    wproj2T = din("wproj2T", (C, C), BF16)
    wfc1T = din("wfc1T", (C, HID), BF16)
    wfc2T = din("wfc2T", (HID, C), BF16)
    if mode == "causal":
        pmask = din("pmask", (128, 2 * 64), BF16)   # {0,1} mult mask
    if mode == "dense":
        dmask = din("dmask", (NP * 128, 2 * R), BF16)
    if not skip_gb:
        lng = {k: din("g_" + k, (HID if k == "mln2" else C,))
               for k in ("ln1", "aln2", "a2ln", "mln1", "mln2")}
        lnb = {k: din("b_" + k, (HID if k == "mln2" else C,))
               for k in ("ln1", "aln2", "a2ln", "mln1", "mln2")}
    if not skip_bias:
        projb = din("projb", (C,))
        proj2b = din("proj2b", (C,))
        fc1b = din("fc1b", (HID,))
        fc2b = din("fc2b", (C,))
    out_my = nc.dram_tensor("out_my", [R, C], F32, kind="ExternalOutput").ap()

    def bcast(vec_ap, n, parts=128):
        return bass.AP(tensor=vec_ap.tensor, offset=vec_ap.offset,
                       ap=[[0, parts]] + vec_ap.ap)

    with tile.TileContext(nc) as tc:
        with tc.tile_pool(name="singles", bufs=1) as singles, \
             tc.tile_pool(name="stats", bufs=4) as stats, \
             tc.tile_pool(name="resid", bufs=1) as resid:

            ident = singles.tile([128, 128], BF16, name="ident", tag="ident")
            make_identity(nc, ident)
            eps_t = singles.tile([128, 1], F32, name="eps", tag="eps")
            nc.vector.memset(eps_t, EPS)

            if mode == "causal":
                pm_t = singles.tile([128, 2, 64], BF16, name="pmask", tag="pmask")
                nc.sync.dma_start(out=pm_t.rearrange("p a b -> p (a b)"), in_=pmask)
            if mode == "dense":
                dm_t = [singles.tile([128, 2, R], BF16, name=f"dmask{j}",
                                     tag=f"dmask{j}") for j in range(NP)]
                for j in range(NP):
                    nc.sync.dma_start(
                        out=dm_t[j].rearrange("p a b -> p (a b)"),
                        in_=dmask[j * 128:(j + 1) * 128, :])

            gb_tiles = {}
            if not skip_gb:
                for k in ("ln1", "aln2", "a2ln", "mln1", "mln2"):
                    d = HID if k == "mln2" else C
                    gt = singles.tile([128, d], F32, name=f"g_{k}", tag=f"g_{k}")
                    bt = singles.tile([128, d], F32, name=f"b_{k}", tag=f"b_{k}")
                    nc.sync.dma_start(out=gt, in_=bcast(lng[k], d))
                    nc.sync.dma_start(out=bt, in_=bcast(lnb[k], d))
                    gb_tiles[k] = (gt, bt)
            bias_tiles = {}
            if not skip_bias:
                _bias_aps = {"projb": projb, "proj2b": proj2b,
                             "fc1b": fc1b, "fc2b": fc2b}
                for k, d in (("projb", C), ("proj2b", C), ("fc1b", HID),
                             ("fc2b", C)):
                    t = singles.tile([128, d], F32, name=k, tag=k)
                    nc.sync.dma_start(out=t, in_=bcast(_bias_aps[k], d))
                    bias_tiles[k] = t

            # ---- helpers ---------------------------------------------------
            def ln_apply(h_out, x_in, d, key, apply_eng="act", st_in=None):
                """LayerNorm of x_in [128, d] -> h_out (any dtype)."""
                if st_in is not None:
                    st = st_in
                else:
                    nsub = max(1, d // 512)
                    st = stats.tile([128, nsub, 6], F32, name="bnst", tag="bnst")
                    if nsub > 1:
                        xr = x_in.rearrange("p (s q) -> p s q", s=nsub)
                        for s in range(nsub):
                            nc.vector.bn_stats(out=st[:, s, :], in_=xr[:, s, :])
                    else:
                        nc.vector.bn_stats(out=st[:, 0, :], in_=x_in)
                mv = stats.tile([128, 2], F32, name="bnmv", tag="bnmv")
                nc.vector.bn_aggr(out=mv, in_=st)
                sd = stats.tile([128, 1], F32, name="bnsd", tag="bnsd")
                nc.scalar.activation(out=sd, in_=mv[:, 1:2], func=AF.Sqrt,
                                     bias=eps_t)
                ri = stats.tile([128, 1], F32, name="bnri", tag="bnri")
                nc.vector.reciprocal(out=ri, in_=sd)
                nm = stats.tile([128, 1], F32, name="bnnm", tag="bnnm")
                nc.vector.tensor_scalar(out=nm, in0=mv[:, 0:1], scalar1=ri,
                                        scalar2=-1.0, op0=ALU.mult, op1=ALU.mult)
                if apply_eng == "act":
                    nc.scalar.activation(out=h_out, in_=x_in, func=AF.Identity,
                                         bias=nm, scale=ri)
                else:
                    nc.vector.tensor_scalar(out=h_out, in0=x_in, scalar1=ri,
                                            scalar2=nm, op0=ALU.mult, op1=ALU.add)
                if not skip_gb:
                    gt, bt = gb_tiles[key]
                    nc.vector.tensor_tensor(out=h_out, in0=h_out, in1=gt[:, :d],
                                            op=ALU.mult)
                    nc.vector.tensor_tensor(out=h_out, in0=h_out, in1=bt[:, :d],
                                            op=ALU.add)

            def transpose_to(pp, dst, src_bf16, nblk, copy_eng):
                """src [128, nblk*128] bf16 -> dst AP [128, nblk, 128]."""
                ptb = pp.tile([128, nblk, 128], BF16, name="tpb", tag="tpb")
                for k in range(nblk):
                    nc.tensor.transpose(
                        ptb[:, k, :], src_bf16[:, k * 128:(k + 1) * 128], ident)
                copy_eng(out=dst, in_=ptb)

            def attention(qT_t, kT_t, v_p, consume_head, causal, lgp, opsp, ptp,
                          msk):
                """Per head: softmax(qk + mask) @ v. consume_head(h, o_ps)."""
                for h in range(H):
                    po = (h % 2) * 64
                    m = h // 2
                    o_ps = opsp.tile([65, R], F32, name="ops", tag="ops")
                    for jp in range(NP):
                        r0 = 64 * jp if causal else 0
                        nj = R - r0
                        lg = lgp.tile([128, 2, R], F32, name="lg", tag="lg")
                        for b in range(2):
                            nc.tensor.matmul(
                                lg[:, b, 0:nj],
                                kT_t[m][po:po + 64,
                                        (2 * jp + b) * 128:(2 * jp + b + 1) * 128],
                                qT_t[po:po + 64, m, r0:R])
                        pt = ptp.tile([128, 2, R], BF16, name="pt", tag="pt")
                        nc.scalar.activation(out=pt[:, :, 0:nj], in_=lg[:, :, 0:nj],
                                             func=AF.Exp)
                        if causal:
                            nc.vector.tensor_tensor(out=pt[:, :, 0:64],
                                                    in0=pt[:, :, 0:64],
                                                    in1=msk, op=ALU.mult)
                        elif mode == "dense":
                            nc.vector.tensor_tensor(out=pt[:, :, 0:nj],
                                                    in0=pt[:, :, 0:nj],
                                                    in1=dm_t[jp][:, :, 0:nj],
                                                    op=ALU.mult)
                        for b in range(2):
                            nc.tensor.matmul(
                                o_ps[:, r0:R],
                                v_p[jp][:, b, h * 65:h * 65 + 65],
                                pt[:, b, 0:nj],
                                start=(jp == 0 and b == 0),
                                stop=(jp == NP - 1 and b == 1))
                    consume_head(h, o_ps)

            def consume_row_head(o_dst, btp):
                def fn(h, o_ps):
                    ot = stats.tile([65, R], BF16, name="ot", tag="ot")
                    nc.vector.tensor_copy(out=ot, in_=o_ps)
                    for t in range(RT):
                        tp = btp.tile([128, 65], BF16, name="otp", tag="otp")
                        nc.tensor.transpose(tp, ot[:, t * 128:(t + 1) * 128],
                                            ident[0:65, 0:65])
                        ri = stats.tile([128, 1], F32, name="osum", tag="osum")
                        nc.vector.reciprocal(out=ri, in_=tp[:, 64:65])
                        nc.vector.tensor_scalar(
                            out=o_dst[t][:, h * 64:(h + 1) * 64],
                            in0=tp[:, 0:64], scalar1=ri,
                            scalar2=None, op0=ALU.mult)
                return fn

            def rows_matmul(pp, wp, lhsT, w_ap, dout, kdim, consume, wtag):
                """out[rows, dout] = act @ W (lhsT packed [128, kdim//128, R])."""
                nkt = kdim // 128
                for nch in range(dout // 512):
                    pss = [pp.tile([128, 512], F32, name=f"mm{rt}",
                                   tag=f"mm{rt}") for rt in range(RT)]
                    for K in range(nkt):
                        wt = wp.tile([128, 512], BF16, name=wtag, tag=wtag)
                        nc.sync.dma_start(
                            out=wt, in_=w_ap[K * 128:(K + 1) * 128,
                                             nch * 512:(nch + 1) * 512])
                        for rt in range(RT):
                            nc.tensor.matmul(
                                pss[rt], lhsT[:, K, rt * 128:(rt + 1) * 128],
                                wt, start=(K == 0), stop=(K == nkt - 1))
                    for rt in range(RT):
                        consume(rt, nch, pss[rt])

            # ================================================================
            # long-lived tiles: softmax outputs only (x1 lives in a C..E pool)
            # ================================================================
            o_sb = [resid.tile([128, C], BF16, name=f"osb{t}", tag=f"osb{t}")
                    for t in range(RT)]
            o2_sb = [resid.tile([128, C], BF16, name=f"o2sb{t}", tag=f"osb{t}")
                     for t in range(RT)]

            # pools by lifetime (manually scoped, non-LIFO):
            #   akv: A..B (self q/k/v), kv2: B..D (cross k/v + q2),
            #   x1p: C..E (x1 residual)
            _cm_akv = tc.tile_pool(name="akv", bufs=1, side="right")
            akv = _cm_akv.__enter__()
            if True:
                qT = akv.tile([128, KIN, R], BF16, name="qT", tag="qT")
                kT = [akv.tile([128, N], BF16, name=f"kT{m}", tag=f"kT{m}")
                      for m in range(KIN)]
                v_p = [akv.tile([128, 2, H * 65], BF16, name=f"v{j}", tag=f"v{j}")
                       for j in range(NP)]

                # ---- stage A: ln1 + self QKV builds ------------------------
                with tc.tile_pool(name="ahb", bufs=16) as ahb, \
                     tc.tile_pool(name="ahmy", bufs=1) as ahmy, \
                     tc.tile_pool(name="awork", bufs=2) as awk, \
                     tc.tile_pool(name="apsT", bufs=2, space="PSUM") as apsT, \
                     tc.tile_pool(name="apsB", bufs=2, space="PSUM") as apsB, \
                     tc.tile_pool(name="awt", bufs=1) as awt:
                    hmyT = ahmy.tile([128, KIN, R], BF16, name="hmyT", tag="hmyT")

                    # all rows first: ln1 -> transposed tiles hb[t]
                    hb = []
                    for t in range(KB):
                        hb.append(ahb.tile([128, KIN, 128], BF16, name=f"hb{t}",
                                           tag="hb"))
                        xf = awk.tile([128, C], F32, name="xfull", tag="xfull")
                        nc.sync.dma_start(out=xf,
                                          in_=x_full[t * 128:(t + 1) * 128, :])
                        hf = awk.tile([128, C], BF16, name="hfull", tag="hfull")
                        ln_apply(hf, xf, C, "ln1",
                                 apply_eng=("act" if t % 2 else "dve"))
                        ce = ((lambda out, in_:
                               nc.vector.tensor_copy(out=out, in_=in_))
                              if t % 2 == 0 else
                              (lambda out, in_: nc.scalar.copy(out=out, in_=in_)))
                        transpose_to(apsT, hb[t], hf, KIN, ce)

                    # own rows (for q): ln1(x_my) transposed
                    for t in range(RT):
                        xm = awk.tile([128, C], F32, name="xm", tag="xfull")
                        nc.sync.dma_start(out=xm,
                                          in_=x_my[t * 128:(t + 1) * 128, :])
                        hm = awk.tile([128, C], BF16, name="hmy", tag="hmy")
                        ln_apply(hm, xm, C, "ln1", apply_eng="act")
                        transpose_to(apsT, hmyT[:, :, t * 128:(t + 1) * 128], hm,
                                     KIN, (lambda out, in_:
                                           nc.vector.tensor_copy(out=out,
                                                                 in_=in_)))

                    # q build
                    for mh in range(2):
                        wts = []
                        for K in range(KIN):
                            wt = awt.tile([128, 512], BF16, name=f"wq{K}",
                                          tag=f"wq{K}", bufs=1)
                            nc.sync.dma_start(
                                out=wt, in_=wqkT[K * 128:(K + 1) * 128,
                                                 mh * 512:(mh + 1) * 512])
                            wts.append(wt)
                        for mm in range(4):
                            m = mh * 4 + mm
                            ps = apsB.tile([128, R], F32, name="qps", tag="qps")
                            for K in range(KIN):
                                nc.tensor.matmul(
                                    ps, wts[K][:, mm * 128:(mm + 1) * 128],
                                    hmyT[:, K, :], start=(K == 0),
                                    stop=(K == KIN - 1))
                            nc.scalar.mul(out=qT[:, m, :], in_=ps, mul=SCALE)

                    # k build: key-chunk outer so early key blocks finish first
                    kwts = []
                    for K in range(KIN):
                        wt = awt.tile([128, C], BF16, name=f"wk{K}",
                                      tag=f"wk{K}", bufs=1)
                        nc.sync.dma_start(out=wt,
                                          in_=wqkT[K * 128:(K + 1) * 128,
                                                   C:2 * C])
                        kwts.append(wt)
                    for n in range(N // 512):
                        for m in range(KIN):
                            ps = apsB.tile([128, 512], F32, name="kps", tag="kps")
                            for tt in range(4):
                                t = 4 * n + tt
                                csl = slice(tt * 128, (tt + 1) * 128)
                                for K in range(KIN):
                                    nc.tensor.matmul(
                                        ps[:, csl],
                                        kwts[K][:, m * 128:(m + 1) * 128],
                                        hb[t][:, K, :],
                                        start=(K == 0), stop=(K == KIN - 1))
                            nc.vector.tensor_copy(
                                out=kT[m][:, n * 512:(n + 1) * 512], in_=ps)

                    # v build
                    for half in range(2):
                        vwts = []
                        for K in range(KIN):
                            wt = awt.tile([128, 512], BF16, name=f"wv{K}",
                                          tag=f"wv{K}", bufs=1)
                            nc.sync.dma_start(
                                out=wt, in_=wvT[K * 128:(K + 1) * 128,
                                                half * 512:(half + 1) * 512])
                            vwts.append(wt)
                        for t in range(KB):
                            ps = apsB.tile([128, 512], F32, name="vps", tag="vps")
                            for K in range(KIN):
                                nc.tensor.matmul(ps, hb[t][:, K, :], vwts[K],
                                                 start=(K == 0),
                                                 stop=(K == KIN - 1))
                            dst = v_p[t // 2].rearrange("p a (h c) -> p a h c",
                                                        c=65)
                            nc.vector.tensor_copy(
                                out=dst[:, t % 2, half * 8:(half + 1) * 8, 0:64],
                                in_=ps.rearrange("p (h c) -> p h c", c=64))
                    for jp in range(NP):
                        dst = v_p[jp].rearrange("p a (h c) -> p a h c", c=65)
                        nc.vector.memset(dst[:, :, :, 64:65], 1.0)

                # ---- stage B: self attention + cross k2/v2 builds ----------
                _cm_kv2 = tc.tile_pool(name="kv2", bufs=1)
                kv2 = _cm_kv2.__enter__()
                q2T = kv2.tile([128, KIN, R], BF16, name="q2T", tag="q2T")
                k2T = [kv2.tile([128, N], BF16, name=f"k2T{m}", tag=f"k2T{m}")
                       for m in range(KIN)]
                v2_p = [kv2.tile([128, 2, H * 65], BF16, name=f"v2{j}",
                                 tag=f"v2{j}") for j in range(NP)]

                _cm_bkv2ps = tc.tile_pool(name="bkv2ps", bufs=2, space="PSUM")
                _cm_bkv2w = tc.tile_pool(name="bkv2w", bufs=1)
                bkv2ps = _cm_bkv2ps.__enter__()
                bkv2w = _cm_bkv2w.__enter__()

                with tc.tile_pool(name="batt", bufs=3) as batt, \
                     tc.tile_pool(name="blg", bufs=2, space="PSUM") as blg, \
                     tc.tile_pool(name="bops", bufs=1, space="PSUM") as bops, \
                     tc.tile_pool(name="btp", bufs=1, space="PSUM") as btp:

                    attention(qT, kT, v_p, consume_row_head(o_sb, btp),
                              causal=(mode == "causal"), lgp=blg, opsp=bops,
                              ptp=batt, msk=pm_t if mode == "causal" else None)

                _cm_akv.__exit__(None, None, None)  # self K/V dead

                # ---- stage C: ln2 + proj + residual; a2ln + q2 build -------
                _cm_x1p = tc.tile_pool(name="x1p", bufs=1, side="right")
                x1p = _cm_x1p.__enter__()
                x1_my = [x1p.tile([128, C], F32, name=f"x1my{t}", tag=f"x1my{t}")
                         for t in range(RT)]

                with tc.tile_pool(name="cw", bufs=2) as cw, \
                     tc.tile_pool(name="colnT", bufs=1) as colnT, \
                     tc.tile_pool(name="cwt", bufs=4) as cwt, \
                     tc.tile_pool(name="cps", bufs=1, space="PSUM") as cps, \
                     tc.tile_pool(name="cps1", bufs=1, space="PSUM") as cps1:
                    olnT = colnT.tile([128, KIN, R], BF16, name="olnT",
                                      tag="olnT")
                    h2T = colnT.tile([128, KIN, R], BF16, name="h2T", tag="h2T")
                    xm_c = [cw.tile([128, C], F32, name=f"xmc{t}", tag="xmc",
                                    bufs=4) for t in range(RT)]
                    for t in range(RT):
                        nc.sync.dma_start(out=xm_c[t],
                                          in_=x_my[t * 128:(t + 1) * 128, :])
                        oln = cw.tile([128, C], BF16, name="oln", tag="oln")
                        ln_apply(oln, o_sb[t], C, "aln2", apply_eng="act")
                        transpose_to(cps, olnT[:, :, t * 128:(t + 1) * 128], oln,
                                     KIN, (lambda out, in_:
                                           nc.vector.tensor_copy(out=out,
                                                                 in_=in_)))

                    def consume_proj(rt, nch, ps):
                        sl = slice(nch * 512, (nch + 1) * 512)
                        if not skip_bias:
                            nc.vector.tensor_tensor(
                                out=ps, in0=ps,
                                in1=bias_tiles["projb"][:, sl], op=ALU.add)
                        nc.vector.tensor_tensor(out=x1_my[rt][:, sl], in0=ps,
                                                in1=xm_c[rt][:, sl], op=ALU.add)

                    rows_matmul(cps1, cwt, olnT, wprojT, C, C, consume_proj, "pw")

                    for t in range(RT):
                        h2 = cw.tile([128, C], BF16, name="h2", tag="h2")
                        ln_apply(h2, x1_my[t], C, "a2ln", apply_eng="act")
                        transpose_to(cps, h2T[:, :, t * 128:(t + 1) * 128], h2,
                                     KIN, (lambda out, in_:
                                           nc.scalar.copy(out=out, in_=in_)))

                    # q2 build
                    for mh in range(2):
                        wts = []
                        for K in range(KIN):
                            wt = cwt.tile([128, 512], BF16, name=f"wq2{K}",
                                          tag=f"wq2{K}", bufs=1)
                            nc.sync.dma_start(
                                out=wt, in_=wq2T[K * 128:(K + 1) * 128,
                                                 mh * 512:(mh + 1) * 512])
                            wts.append(wt)
                        for mm in range(4):
                            m = mh * 4 + mm
                            ps = cps1.tile([128, R], F32, name="q2ps", tag="q2ps")
                            for K in range(KIN):
                                nc.tensor.matmul(
                                    ps, wts[K][:, mm * 128:(mm + 1) * 128],
                                    h2T[:, K, :], start=(K == 0),
                                    stop=(K == KIN - 1))
                            nc.scalar.mul(out=q2T[:, m, :], in_=ps, mul=SCALE)

                # cross k2/v2 builds: emitted after C so C's critical
                # DMAs take queue priority; matmuls still fill B/C PE gaps
                if True:
                    # k2 build: n-chunk outer (y chunks loaded once), resident
                    # k-half weights
                    k2wts = []
                    for K in range(KIN):
                        wt = bkv2w.tile([128, C], BF16, name=f"wk2{K}",
                                        tag=f"wk2{K}", bufs=1)
                        nc.sync.dma_start(out=wt,
                                          in_=wkv2T[K * 128:(K + 1) * 128, 0:C])
                        k2wts.append(wt)
                    for n in range(N // 512):
                        ycs = []
                        for K in range(KIN):
                            yc = bkv2w.tile([128, 512], BF16, name=f"yc{K}",
                                            tag=f"yc{K}", bufs=1)
                            nc.sync.dma_start(
                                out=yc, in_=yTb[K * 128:(K + 1) * 128,
                                                n * 512:(n + 1) * 512])
                            ycs.append(yc)
                        for m in range(KIN):
                            ps = bkv2ps.tile([128, 512], F32, name="k2ps",
                                             tag="kv2ps")
                            for K in range(KIN):
                                nc.tensor.matmul(
                                    ps, k2wts[K][:, m * 128:(m + 1) * 128],
                                    ycs[K], start=(K == 0), stop=(K == KIN - 1))
                            nc.vector.tensor_copy(
                                out=k2T[m][:, n * 512:(n + 1) * 512], in_=ps)

                    # v2 build: y blocks streamed per key tile
                    for half in range(2):
                        v2wts = []
                        for K in range(KIN):
                            wt = bkv2w.tile([128, 512], BF16, name=f"wv2{K}",
                                            tag=f"wv2{K}", bufs=1)
                            nc.sync.dma_start(
                                out=wt, in_=wkv2T[K * 128:(K + 1) * 128,
                                                  C + half * 512:
                                                  C + (half + 1) * 512])
                            v2wts.append(wt)
                        for t in range(KB):
                            yb = bkv2w.tile([128, KIN, 128], BF16, name="yb",
                                            tag="yb", bufs=2)
                            nc.sync.dma_start(
                                out=yb,
                                in_=yBlk[t * 128:(t + 1) * 128, :]
                                .rearrange("p (K k) -> p K k", k=128))
                            ps = bkv2ps.tile([128, 512], F32, name="v2ps",
                                             tag="kv2ps")
                            for K in range(KIN):
                                nc.tensor.matmul(ps, yb[:, K, :], v2wts[K],
                                                 start=(K == 0),
                                                 stop=(K == KIN - 1))
                            dst = v2_p[t // 2].rearrange("p a (h c) -> p a h c",
                                                         c=65)
                            nc.vector.tensor_copy(
                                out=dst[:, t % 2, half * 8:(half + 1) * 8, 0:64],
                                in_=ps.rearrange("p (h c) -> p h c", c=64))
                    for jp in range(NP):
                        dst = v2_p[jp].rearrange("p a (h c) -> p a h c", c=65)
                        nc.vector.memset(dst[:, :, :, 64:65], 1.0)


                _cm_bkv2w.__exit__(None, None, None)
                _cm_bkv2ps.__exit__(None, None, None)

                # ---- stage D: cross attention ------------------------------
                with tc.tile_pool(name="datt", bufs=3) as datt, \
                     tc.tile_pool(name="dlg", bufs=2, space="PSUM") as dlg, \
                     tc.tile_pool(name="dops", bufs=1, space="PSUM") as dops, \
                     tc.tile_pool(name="dtp", bufs=1, space="PSUM") as dtp:
                    attention(q2T, k2T, v2_p, consume_row_head(o2_sb, dtp),
                              causal=False, lgp=dlg, opsp=dops, ptp=datt,
                              msk=None)

                _cm_kv2.__exit__(None, None, None)  # cross K/V dead

                # ---- stage E: proj2 + residual; MLP ------------------------
                with tc.tile_pool(name="ew", bufs=2) as ew, \
                     tc.tile_pool(name="eh4", bufs=1) as eh4, \
                     tc.tile_pool(name="ewt", bufs=4) as ewt, \
                     tc.tile_pool(name="eps2", bufs=2, space="PSUM") as eps2, \
                     tc.tile_pool(name="eps1", bufs=1, space="PSUM") as eps1:
                    x2_my = [eh4.tile([128, C], F32, name=f"x2my{t}",
                                      tag=f"x2my{t}") for t in range(RT)]
                    o2T = eh4.tile([128, KIN, R], BF16, name="o2T", tag="o2T")
                    for t in range(RT):
                        transpose_to(eps2, o2T[:, :, t * 128:(t + 1) * 128],
                                     o2_sb[t], KIN,
                                     (lambda out, in_:
                                      nc.vector.tensor_copy(out=out, in_=in_)))

                    def consume_proj2(rt, nch, ps):
                        sl = slice(nch * 512, (nch + 1) * 512)
                        if not skip_bias:
                            nc.vector.tensor_tensor(
                                out=ps, in0=ps,
                                in1=bias_tiles["proj2b"][:, sl], op=ALU.add)
                        nc.vector.tensor_tensor(out=x2_my[rt][:, sl], in0=ps,
                                                in1=x1_my[rt][:, sl], op=ALU.add)

                    rows_matmul(eps1, ewt, o2T, wproj2T, C, C, consume_proj2,
                                "p2w")

                    # MLP
                    h3T = eh4.tile([128, KIN, R], BF16, name="h3T", tag="h3T")
                    h4 = [eh4.tile([128, HID], BF16, name=f"h4_{t}",
                                   tag=f"h4_{t}") for t in range(RT)]
                    h5T = eh4.tile([128, HID // 128, R], BF16, name="h5T",
                                   tag="h5T")
                    for t in range(RT):
                        h3 = ew.tile([128, C], BF16, name="h3", tag="h3")
                        ln_apply(h3, x2_my[t], C, "mln1", apply_eng="act")
                        transpose_to(eps2, h3T[:, :, t * 128:(t + 1) * 128], h3,
                                     KIN, (lambda out, in_:
                                           nc.vector.tensor_copy(out=out,
                                                                 in_=in_)))

                    mst = [eh4.tile([128, HID // 512, 6], F32,
                                    name=f"mst{t}", tag=f"mst{t}")
                           for t in range(RT)]

                    def consume_fc1(rt, nch, ps):
                        sl = slice(nch * 512, (nch + 1) * 512)
                        if not skip_bias:
                            nc.vector.tensor_tensor(
                                out=ps, in0=ps,
                                in1=bias_tiles["fc1b"][:, sl], op=ALU.add)
                        nc.scalar.activation(out=h4[rt][:, sl], in_=ps,
                                             func=AF.Gelu)
                        nc.vector.bn_stats(out=mst[rt][:, nch, :],
                                           in_=h4[rt][:, sl])

                    rows_matmul(eps1, ewt, h3T, wfc1T, HID, C, consume_fc1,
                                "f1w")

                    for t in range(RT):
                        h5 = ew.tile([128, HID], BF16, name="h5", tag="h5",
                                     bufs=1)
                        ln_apply(h5, h4[t], HID, "mln2", apply_eng="dve",
                                 st_in=mst[t])
                        for g in range(0, HID // 128, 8):
                            transpose_to(
                                eps2,
                                h5T[:, g:g + 8, t * 128:(t + 1) * 128],
                                h5[:, g * 128:(g + 8) * 128], 8,
                                (lambda out, in_:
                                 nc.vector.tensor_copy(out=out, in_=in_))
                                if (g // 8) % 2 == 0 else
                                (lambda out, in_: nc.scalar.copy(out=out,
                                                                 in_=in_)))

                    def consume_fc2(rt, nch, ps):
                        sl = slice(nch * 512, (nch + 1) * 512)
                        if not skip_bias:
                            nc.vector.tensor_tensor(
                                out=ps, in0=ps,
                                in1=bias_tiles["fc2b"][:, sl], op=ALU.add)
                        x3 = ew.tile([128, 512], F32, name="x3", tag="x3")
                        nc.vector.tensor_tensor(out=x3, in0=ps,
                                                in1=x2_my[rt][:, sl], op=ALU.add)
                        nc.sync.dma_start(
                            out=out_my[rt * 128:(rt + 1) * 128, sl], in_=x3)

                    rows_matmul(eps1, ewt, h5T, wfc2T, C, HID, consume_fc2,
                                "f2w")

                _cm_x1p.__exit__(None, None, None)

    nc.compile()
    return nc


# ---------------------------------------------------------------------------
# host side
# ---------------------------------------------------------------------------

def _host_prep(inputs):
    f32 = np.float32
    x = np.asarray(inputs["x"], f32)
    y = np.asarray(inputs["y"], f32)
    mask = np.asarray(inputs["mask"])[0, 0]  # [N, N] bool

    causal_ref = np.triu(np.ones((N, N), bool), k=1)
    if np.array_equal(mask, causal_ref):
        mode = "causal"
    elif not mask.any():
        mode = "none"
    else:
        mode = "dense"

    gbs = [("a1_ln1_g", "a1_ln1_b"), ("a1_ln2_g", "a1_ln2_b"),
           ("a2_ln_g", "a2_ln_b"), ("m_ln1_g", "m_ln1_b"), ("m_ln2_g", "m_ln2_b")]
    skip_gb = all(
        np.all(np.asarray(inputs[g]) == 1.0) and np.all(np.asarray(inputs[b]) == 0.0)
        for g, b in gbs)
    skip_bias = all(np.all(np.asarray(inputs[k]) == 0.0)
                    for k in ("a1_proj_b", "a2_proj_b", "m_fc1_b", "m_fc2_b"))

    wTb = lambda k: np.ascontiguousarray(
        np.asarray(inputs[k], f32).T).astype(BF16NP)
    shared = {
        "wqkT": wTb("a1_qk_w"),
        "wvT": wTb("a1_v_w"),
        "wq2T": wTb("a2_q_w"),
        "wkv2T": wTb("a2_kv_w"),
        "wprojT": wTb("a1_proj_w"),
        "wproj2T": wTb("a2_proj_w"),
        "wfc1T": wTb("m_fc1_w"),
        "wfc2T": wTb("m_fc2_w"),
    }
    if not skip_gb:
        keymap = {"ln1": ("a1_ln1_g", "a1_ln1_b"), "aln2": ("a1_ln2_g", "a1_ln2_b"),
                  "a2ln": ("a2_ln_g", "a2_ln_b"), "mln1": ("m_ln1_g", "m_ln1_b"),
                  "mln2": ("m_ln2_g", "m_ln2_b")}
        for k, (gk, bk) in keymap.items():
            shared["g_" + k] = np.asarray(inputs[gk], f32)
            shared["b_" + k] = np.asarray(inputs[bk], f32)
    if not skip_bias:
        shared["projb"] = np.asarray(inputs["a1_proj_b"], f32)
        shared["proj2b"] = np.asarray(inputs["a2_proj_b"], f32)
        shared["fc1b"] = np.asarray(inputs["m_fc1_b"], f32)
        shared["fc2b"] = np.asarray(inputs["m_fc2_b"], f32)

    in_maps = []
    for core in range(G * CPG):
        g, c = core // CPG, core % CPG
        m = dict(shared)
        m["x_my"] = np.ascontiguousarray(x[g, c::CPG])
        m["x_full"] = np.ascontiguousarray(x[g])
        yT_g = np.ascontiguousarray(y[g].T).astype(BF16NP)
        m["yTb"] = yT_g
        m["yBlk"] = np.ascontiguousarray(
            yT_g.reshape(KIN, 128, KB, 128).transpose(2, 1, 0, 3)
            .reshape(KB * 128, KIN * 128))
        if mode == "causal":
            # pair mask [128 keys, 2 blocks, 64 range-rows]: block b allows
            # key kk iff kk <= c + 4*d - 128*b  (d = row offset in range)
            kk = np.arange(128)[:, None, None]
            bb = np.arange(2)[None, :, None]
            dd = np.arange(64)[None, None, :]
            pm = (kk <= c + 4 * dd - 128 * bb).astype(f32)
            m["pmask"] = pm.reshape(128, 128).astype(BF16NP)
        if mode == "dense":
            sub = (~mask[c::CPG, :]).astype(f32)  # [R, N] rows x keys
            dm = np.empty((NP * 128, 2 * R), f32)
            for jp in range(NP):
                for b in range(2):
                    blk = sub[:, (2 * jp + b) * 128:(2 * jp + b + 1) * 128].T
                    dm[jp * 128:(jp + 1) * 128, b * R:(b + 1) * R] = blk
            m["dmask"] = dm.astype(BF16NP)
        in_maps.append(m)
    return mode, skip_gb, skip_bias, in_maps


def _assemble(results, dtype):
    out = np.empty((B, N, C), np.float32)
    for core in range(G * CPG):
        g, c = core // CPG, core % CPG
        out[g, c::CPG] = results[core]["out_my"]
    return out.astype(dtype, copy=False)


def get_program(inputs):
    """Build (or fetch cached) program + per-core input maps for these inputs."""
    mode, skip_gb, skip_bias, in_maps = _host_prep(inputs)
    key = (mode, skip_gb, skip_bias)
    if key not in _CACHE:
        _CACHE[key] = _build(mode, skip_gb, skip_bias)
    return _CACHE[key], in_maps


def kernel(**inputs):
    from concourse import bass_utils

    nc, in_maps = get_program(inputs)
    res = bass_utils.run_bass_kernel_spmd(nc, in_maps, core_ids=list(range(8)))
    return _assemble(res.results, np.asarray(inputs["x"]).dtype)


# revision 33
# speedup vs baseline: 1.0248x; 1.0248x over previous
"""Trainium2 Bass kernel for a transformer decoder block (self-attn + cross-attn + MLP).

Sharding: 8 cores = 2 batch groups x 4 cores. Within a group, core c owns
rows r = c (mod 4) of its batch (strided rows balance causal attention work
while keeping the compiled program identical across cores). K/V are computed
replicated within a group (no collectives; cores are fully independent).

v3 (all-bf16): bf16 weights and activations everywhere (halves DMA and
transpose cost vs f32); K/V builds consume transposed-h tiles directly from
a rolling SBUF window (no DRAM staging); key blocks processed in pairs to
halve exp/mask instruction count; cross K/V built from DRAM-streamed yT and
emitted after the self-attention inner loop so the list scheduler fills PE
gaps under the ACT-bound exp phase; h4 staged in SBUF.
"""

import os
import sys

for _p in ("/opt/trn_rl_repo", "/root/.axon_site/_ro/trn_rl_repo"):
    if os.path.isdir(_p) and _p not in sys.path:
        sys.path.insert(0, _p)

import numpy as np
import ml_dtypes

BF16NP = ml_dtypes.bfloat16

B, N, C, H, Y_DIM, HID = 2, 2048, 1024, 16, 1024, 4096
HD = C // H
SCALE = HD ** -0.5
EPS = 1e-5

G = 2           # batch groups
CPG = 4         # cores per group
R = N // CPG    # rows per core (512)
RT = R // 128   # row tiles per core (4)
KB = N // 128   # key blocks (16)
NP = KB // 2    # key block pairs (8)
KIN = C // 128  # contraction tiles for C (8)

_CACHE = {}


# ---------------------------------------------------------------------------
# program builder
# ---------------------------------------------------------------------------

def _build(mode, skip_gb, skip_bias):
    """mode: 'causal' | 'none' | 'dense'"""
    import concourse.bass as bass
    import concourse.mybir as mybir
    import concourse.tile as tile
    from concourse import bacc
    from concourse.masks import make_identity

    dt = mybir.dt
    F32, BF16 = dt.float32, dt.bfloat16
    AF = mybir.ActivationFunctionType
    ALU = mybir.AluOpType

    nc = bacc.Bacc("TRN2", target_bir_lowering=False, debug=False, num_devices=8)

    # ---- DRAM I/O ----------------------------------------------------------
    def din(name, shape, dtype=None):
        return nc.dram_tensor(name, list(shape), dtype or F32,
                              kind="ExternalInput").ap()

    x_my = din("x_my", (R, C))
    x_full = din("x_full", (N, C))
    yTb = din("yTb", (Y_DIM, N), BF16)
    yBlk = din("yBlk", (KB * 128, KIN * 128), BF16)
    wqkT = din("wqkT", (C, 2 * C), BF16)     # [in, out] cols 0:C q, C:2C k
    wvT = din("wvT", (C, C), BF16)
    wq2T = din("wq2T", (C, C), BF16)
    wkv2T = din("wkv2T", (Y_DIM, 2 * C), BF16)  # cols 0:C k, C:2C v
    wprojT = din("wprojT", (C, C), BF16)
    wproj2T = din("wproj2T", (C, C), BF16)
    wfc1T = din("wfc1T", (C, HID), BF16)
    wfc2T = din("wfc2T", (HID, C), BF16)
    if mode == "causal":
        pmask = din("pmask", (128, 2 * 64), BF16)   # {0,1} mult mask
    if mode == "dense":
        dmask = din("dmask", (NP * 128, 2 * R), BF16)
    if not skip_gb:
        lng = {k: din("g_" + k, (HID if k == "mln2" else C,))
               for k in ("ln1", "aln2", "a2ln", "mln1", "mln2")}
        lnb = {k: din("b_" + k, (HID if k == "mln2" else C,))
               for k in ("ln1", "aln2", "a2ln", "mln1", "mln2")}
    if not skip_bias:
        projb = din("projb", (C,))
        proj2b = din("proj2b", (C,))
        fc1b = din("fc1b", (HID,))
        fc2b = din("fc2b", (C,))
    out_my = nc.dram_tensor("out_my", [R, C], F32, kind="ExternalOutput").ap()

    def bcast(vec_ap, n, parts=128):
        return bass.AP(tensor=vec_ap.tensor, offset=vec_ap.offset,
                       ap=[[0, parts]] + vec_ap.ap)

    with tile.TileContext(nc) as tc:
        with tc.tile_pool(name="singles", bufs=1) as singles, \
             tc.tile_pool(name="stats", bufs=4) as stats, \
             tc.tile_pool(name="resid", bufs=1) as resid:

            ident = singles.tile([128, 128], BF16, name="ident", tag="ident")
            make_identity(nc, ident)
            eps_t = singles.tile([128, 1], F32, name="eps", tag="eps")
            nc.vector.memset(eps_t, EPS)

            if mode == "causal":
                pm_t = singles.tile([128, 2, 64], BF16, name="pmask", tag="pmask")
                nc.sync.dma_start(out=pm_t.rearrange("p a b -> p (a b)"), in_=pmask)
            if mode == "dense":
                dm_t = [singles.tile([128, 2, R], BF16, name=f"dmask{j}",
                                     tag=f"dmask{j}") for j in range(NP)]
                for j in range(NP):
                    nc.sync.dma_start(
                        out=dm_t[j].rearrange("p a b -> p (a b)"),
                        in_=dmask[j * 128:(j + 1) * 128, :])

            gb_tiles = {}
            if not skip_gb:
                for k in ("ln1", "aln2", "a2ln", "mln1", "mln2"):
                    d = HID if k == "mln2" else C
                    gt = singles.tile([128, d], F32, name=f"g_{k}", tag=f"g_{k}")
                    bt = singles.tile([128, d], F32, name=f"b_{k}", tag=f"b_{k}")
                    nc.sync.dma_start(out=gt, in_=bcast(lng[k], d))
                    nc.sync.dma_start(out=bt, in_=bcast(lnb[k], d))
                    gb_tiles[k] = (gt, bt)
            bias_tiles = {}
            if not skip_bias:
                _bias_aps = {"projb": projb, "proj2b": proj2b,
                             "fc1b": fc1b, "fc2b": fc2b}
                for k, d in (("projb", C), ("proj2b", C), ("fc1b", HID),
                             ("fc2b", C)):
                    t = singles.tile([128, d], F32, name=k, tag=k)
                    nc.sync.dma_start(out=t, in_=bcast(_bias_aps[k], d))
                    bias_tiles[k] = t

            # ---- helpers ---------------------------------------------------
            def ln_apply(h_out, x_in, d, key, apply_eng="act", st_in=None):
                """LayerNorm of x_in [128, d] -> h_out (any dtype)."""
                if st_in is not None:
                    st = st_in
                else:
                    nsub = max(1, d // 512)
                    st = stats.tile([128, nsub, 6], F32, name="bnst", tag="bnst")
                    if nsub > 1:
                        xr = x_in.rearrange("p (s q) -> p s q", s=nsub)
                        for s in range(nsub):
                            nc.vector.bn_stats(out=st[:, s, :], in_=xr[:, s, :])
                    else:
                        nc.vector.bn_stats(out=st[:, 0, :], in_=x_in)
                mv = stats.tile([128, 2], F32, name="bnmv", tag="bnmv")
                nc.vector.bn_aggr(out=mv, in_=st)
                sd = stats.tile([128, 1], F32, name="bnsd", tag="bnsd")
                nc.scalar.activation(out=sd, in_=mv[:, 1:2], func=AF.Sqrt,
                                     bias=eps_t)
                ri = stats.tile([128, 1], F32, name="bnri", tag="bnri")
                nc.vector.reciprocal(out=ri, in_=sd)
                nm = stats.tile([128, 1], F32, name="bnnm", tag="bnnm")
                nc.vector.tensor_scalar(out=nm, in0=mv[:, 0:1], scalar1=ri,
                                        scalar2=-1.0, op0=ALU.mult, op1=ALU.mult)
                if apply_eng == "act":
                    nc.scalar.activation(out=h_out, in_=x_in, func=AF.Identity,
                                         bias=nm, scale=ri)
                else:
                    nc.vector.tensor_scalar(out=h_out, in0=x_in, scalar1=ri,
                                            scalar2=nm, op0=ALU.mult, op1=ALU.add)
                if not skip_gb:
                    gt, bt = gb_tiles[key]
                    nc.vector.tensor_tensor(out=h_out, in0=h_out, in1=gt[:, :d],
                                            op=ALU.mult)
                    nc.vector.tensor_tensor(out=h_out, in0=h_out, in1=bt[:, :d],
                                            op=ALU.add)

            def transpose_to(pp, dst, src_bf16, nblk, copy_eng):
                """src [128, nblk*128] bf16 -> dst AP [128, nblk, 128]."""
                ptb = pp.tile([128, nblk, 128], BF16, name="tpb", tag="tpb")
                for k in range(nblk):
                    nc.tensor.transpose(
                        ptb[:, k, :], src_bf16[:, k * 128:(k + 1) * 128], ident)
                copy_eng(out=dst, in_=ptb)

            def attention(qT_t, kT_t, v_p, consume_head, causal, lgp, opsp, ptp,
                          msk):
                """Per head: softmax(qk + mask) @ v. consume_head(h, o_ps)."""
                for h in range(H):
                    po = (h % 2) * 64
                    m = h // 2
                    o_ps = opsp.tile([65, R], F32, name="ops", tag="ops")
                    for jp in range(NP):
                        r0 = 64 * jp if causal else 0
                        nj = R - r0
                        lg = lgp.tile([128, 2, R], F32, name="lg", tag="lg")
                        for b in range(2):
                            nc.tensor.matmul(
                                lg[:, b, 0:nj],
                                kT_t[m][po:po + 64,
                                        (2 * jp + b) * 128:(2 * jp + b + 1) * 128],
                                qT_t[po:po + 64, m, r0:R])
                        pt = ptp.tile([128, 2, R], BF16, name="pt", tag="pt")
                        nc.scalar.activation(out=pt[:, :, 0:nj], in_=lg[:, :, 0:nj],
                                             func=AF.Exp)
                        if causal:
                            nc.vector.tensor_tensor(out=pt[:, :, 0:64],
                                                    in0=pt[:, :, 0:64],
                                                    in1=msk, op=ALU.mult)
                        elif mode == "dense":
                            nc.vector.tensor_tensor(out=pt[:, :, 0:nj],
                                                    in0=pt[:, :, 0:nj],
                                                    in1=dm_t[jp][:, :, 0:nj],
                                                    op=ALU.mult)
                        for b in range(2):
                            nc.tensor.matmul(
                                o_ps[:, r0:R],
                                v_p[jp][:, b, h * 65:h * 65 + 65],
                                pt[:, b, 0:nj],
                                start=(jp == 0 and b == 0),
                                stop=(jp == NP - 1 and b == 1))
                    consume_head(h, o_ps)

            def consume_row_head(o_dst, btp):
                def fn(h, o_ps):
                    ot = stats.tile([65, R], BF16, name="ot", tag="ot")
                    nc.vector.tensor_copy(out=ot, in_=o_ps)
                    for t in range(RT):
                        tp = btp.tile([128, 65], BF16, name="otp", tag="otp")
                        nc.tensor.transpose(tp, ot[:, t * 128:(t + 1) * 128],
                                            ident[0:65, 0:65])
                        ri = stats.tile([128, 1], F32, name="osum", tag="osum")
                        nc.vector.reciprocal(out=ri, in_=tp[:, 64:65])
                        nc.vector.tensor_scalar(
                            out=o_dst[t][:, h * 64:(h + 1) * 64],
                            in0=tp[:, 0:64], scalar1=ri,
                            scalar2=None, op0=ALU.mult)
                return fn

            def rows_matmul(pp, wp, lhsT, w_ap, dout, kdim, consume, wtag):
                """out[rows, dout] = act @ W (lhsT packed [128, kdim//128, R])."""
                nkt = kdim // 128
                for nch in range(dout // 512):
                    pss = [pp.tile([128, 512], F32, name=f"mm{rt}",
                                   tag=f"mm{rt}") for rt in range(RT)]
                    for K in range(nkt):
                        wt = wp.tile([128, 512], BF16, name=wtag, tag=wtag)
                        nc.sync.dma_start(
                            out=wt, in_=w_ap[K * 128:(K + 1) * 128,
                                             nch * 512:(nch + 1) * 512])
                        for rt in range(RT):
                            nc.tensor.matmul(
                                pss[rt], lhsT[:, K, rt * 128:(rt + 1) * 128],
                                wt, start=(K == 0), stop=(K == nkt - 1))
                    for rt in range(RT):
                        consume(rt, nch, pss[rt])

            # ================================================================
            # long-lived tiles: softmax outputs only (x1 lives in a C..E pool)
            # ================================================================
            o_sb = [resid.tile([128, C], BF16, name=f"osb{t}", tag=f"osb{t}")
                    for t in range(RT)]
            o2_sb = [resid.tile([128, C], BF16, name=f"o2sb{t}", tag=f"osb{t}")
                     for t in range(RT)]

            # pools by lifetime (manually scoped, non-LIFO):
            #   akv: A..B (self q/k/v), kv2: B..D (cross k/v + q2),
            #   x1p: C..E (x1 residual)
            _cm_akv = tc.tile_pool(name="akv", bufs=1, side="right")
            akv = _cm_akv.__enter__()
            if True:
                qT = akv.tile([128, KIN, R], BF16, name="qT", tag="qT")
                kT = [akv.tile([128, N], BF16, name=f"kT{m}", tag=f"kT{m}")
                      for m in range(KIN)]
                v_p = [akv.tile([128, 2, H * 65], BF16, name=f"v{j}", tag=f"v{j}")
                       for j in range(NP)]

                # ---- stage A: ln1 + self QKV builds ------------------------
                with tc.tile_pool(name="ahb", bufs=16) as ahb, \
                     tc.tile_pool(name="ahmy", bufs=1) as ahmy, \
                     tc.tile_pool(name="awork", bufs=2) as awk, \
                     tc.tile_pool(name="apsT", bufs=2, space="PSUM") as apsT, \
                     tc.tile_pool(name="apsB", bufs=2, space="PSUM") as apsB, \
                     tc.tile_pool(name="awt", bufs=1) as awt:
                    hmyT = ahmy.tile([128, KIN, R], BF16, name="hmyT", tag="hmyT")

                    # all rows first: ln1 -> transposed tiles hb[t]
                    hb = []
                    for t in range(KB):
                        hb.append(ahb.tile([128, KIN, 128], BF16, name=f"hb{t}",
                                           tag="hb"))
                        xf = awk.tile([128, C], F32, name="xfull", tag="xfull")
                        nc.sync.dma_start(out=xf,
                                          in_=x_full[t * 128:(t + 1) * 128, :])
                        hf = awk.tile([128, C], BF16, name="hfull", tag="hfull")
                        ln_apply(hf, xf, C, "ln1",
                                 apply_eng=("act" if t % 2 else "dve"))
                        ce = ((lambda out, in_:
                               nc.vector.tensor_copy(out=out, in_=in_))
                              if t % 2 == 0 else
                              (lambda out, in_: nc.scalar.copy(out=out, in_=in_)))
                        transpose_to(apsT, hb[t], hf, KIN, ce)

                    # own rows (for q): ln1(x_my) transposed
                    for t in range(RT):
                        xm = awk.tile([128, C], F32, name="xm", tag="xfull")
                        nc.sync.dma_start(out=xm,
                                          in_=x_my[t * 128:(t + 1) * 128, :])
                        hm = awk.tile([128, C], BF16, name="hmy", tag="hmy")
                        ln_apply(hm, xm, C, "ln1", apply_eng="act")
                        transpose_to(apsT, hmyT[:, :, t * 128:(t + 1) * 128], hm,
                                     KIN, (lambda out, in_:
                                           nc.vector.tensor_copy(out=out,
                                                                 in_=in_)))

                    # q build
                    for mh in range(2):
                        wts = []
                        for K in range(KIN):
                            wt = awt.tile([128, 512], BF16, name=f"wq{K}",
                                          tag=f"wq{K}", bufs=1)
                            nc.sync.dma_start(
                                out=wt, in_=wqkT[K * 128:(K + 1) * 128,
                                                 mh * 512:(mh + 1) * 512])
                            wts.append(wt)
                        for mm in range(4):
                            m = mh * 4 + mm
                            ps = apsB.tile([128, R], F32, name="qps", tag="qps")
                            for K in range(KIN):
                                nc.tensor.matmul(
                                    ps, wts[K][:, mm * 128:(mm + 1) * 128],
                                    hmyT[:, K, :], start=(K == 0),
                                    stop=(K == KIN - 1))
                            nc.scalar.mul(out=qT[:, m, :], in_=ps, mul=SCALE)

                    # k build: key-chunk outer so early key blocks finish first
                    kwts = []
                    for K in range(KIN):
                        wt = awt.tile([128, C], BF16, name=f"wk{K}",
                                      tag=f"wk{K}", bufs=1)
                        nc.sync.dma_start(out=wt,
                                          in_=wqkT[K * 128:(K + 1) * 128,
                                                   C:2 * C])
                        kwts.append(wt)
                    for n in range(N // 512):
                        for m in range(KIN):
                            ps = apsB.tile([128, 512], F32, name="kps", tag="kps")
                            for tt in range(4):
                                t = 4 * n + tt
                                csl = slice(tt * 128, (tt + 1) * 128)
                                for K in range(KIN):
                                    nc.tensor.matmul(
                                        ps[:, csl],
                                        kwts[K][:, m * 128:(m + 1) * 128],
                                        hb[t][:, K, :],
                                        start=(K == 0), stop=(K == KIN - 1))
                            nc.vector.tensor_copy(
                                out=kT[m][:, n * 512:(n + 1) * 512], in_=ps)

                    # v build
                    for half in range(2):
                        vwts = []
                        for K in range(KIN):
                            wt = awt.tile([128, 512], BF16, name=f"wv{K}",
                                          tag=f"wv{K}", bufs=1)
                            nc.sync.dma_start(
                                out=wt, in_=wvT[K * 128:(K + 1) * 128,
                                                half * 512:(half + 1) * 512])
                            vwts.append(wt)
                        for t in range(KB):
                            ps = apsB.tile([128, 512], F32, name="vps", tag="vps")
                            for K in range(KIN):
                                nc.tensor.matmul(ps, hb[t][:, K, :], vwts[K],
                                                 start=(K == 0),
                                                 stop=(K == KIN - 1))
                            dst = v_p[t // 2].rearrange("p a (h c) -> p a h c",
                                                        c=65)
                            nc.vector.tensor_copy(
                                out=dst[:, t % 2, half * 8:(half + 1) * 8, 0:64],
                                in_=ps.rearrange("p (h c) -> p h c", c=64))
                    for jp in range(NP):
                        dst = v_p[jp].rearrange("p a (h c) -> p a h c", c=65)
                        nc.vector.memset(dst[:, :, :, 64:65], 1.0)

                # ---- stage B: self attention + cross k2/v2 builds ----------
                _cm_kv2 = tc.tile_pool(name="kv2", bufs=1)
                kv2 = _cm_kv2.__enter__()
                q2T = kv2.tile([128, KIN, R], BF16, name="q2T", tag="q2T")
                k2T = [kv2.tile([128, N], BF16, name=f"k2T{m}", tag=f"k2T{m}")
                       for m in range(KIN)]
                v2_p = [kv2.tile([128, 2, H * 65], BF16, name=f"v2{j}",
                                 tag=f"v2{j}") for j in range(NP)]

                _cm_bkv2ps = tc.tile_pool(name="bkv2ps", bufs=2, space="PSUM")
                _cm_bkv2w = tc.tile_pool(name="bkv2w", bufs=1)
                bkv2ps = _cm_bkv2ps.__enter__()
                bkv2w = _cm_bkv2w.__enter__()

                with tc.tile_pool(name="batt", bufs=3) as batt, \
                     tc.tile_pool(name="blg", bufs=2, space="PSUM") as blg, \
                     tc.tile_pool(name="bops", bufs=1, space="PSUM") as bops, \
                     tc.tile_pool(name="btp", bufs=1, space="PSUM") as btp:

                    attention(qT, kT, v_p, consume_row_head(o_sb, btp),
                              causal=(mode == "causal"), lgp=blg, opsp=bops,
                              ptp=batt, msk=pm_t if mode == "causal" else None)

                _cm_akv.__exit__(None, None, None)  # self K/V dead

                # ---- stage C: ln2 + proj + residual; a2ln + q2 build -------
                _cm_x1p = tc.tile_pool(name="x1p", bufs=1, side="right")
                x1p = _cm_x1p.__enter__()
                x1_my = [x1p.tile([128, C], F32, name=f"x1my{t}", tag=f"x1my{t}")
                         for t in range(RT)]

                with tc.tile_pool(name="cw", bufs=2) as cw, \
                     tc.tile_pool(name="colnT", bufs=1) as colnT, \
                     tc.tile_pool(name="cwt", bufs=4) as cwt, \
                     tc.tile_pool(name="cps", bufs=1, space="PSUM") as cps, \
                     tc.tile_pool(name="cps1", bufs=1, space="PSUM") as cps1:
                    olnT = colnT.tile([128, KIN, R], BF16, name="olnT",
                                      tag="olnT")
                    h2T = colnT.tile([128, KIN, R], BF16, name="h2T", tag="h2T")
                    xm_c = [cw.tile([128, C], F32, name=f"xmc{t}", tag="xmc",
                                    bufs=4) for t in range(RT)]
                    for t in range(RT):
                        nc.sync.dma_start(out=xm_c[t],
                                          in_=x_my[t * 128:(t + 1) * 128, :])
                        oln = cw.tile([128, C], BF16, name="oln", tag="oln")
                        ln_apply(oln, o_sb[t], C, "aln2", apply_eng="act")
                        transpose_to(cps, olnT[:, :, t * 128:(t + 1) * 128], oln,
                                     KIN, (lambda out, in_:
                                           nc.vector.tensor_copy(out=out,
                                                                 in_=in_)))

                    def consume_proj(rt, nch, ps):
                        sl = slice(nch * 512, (nch + 1) * 512)
                        if not skip_bias:
                            nc.vector.tensor_tensor(
                                out=ps, in0=ps,
                                in1=bias_tiles["projb"][:, sl], op=ALU.add)
                        nc.vector.tensor_tensor(out=x1_my[rt][:, sl], in0=ps,
                                                in1=xm_c[rt][:, sl], op=ALU.add)

                    rows_matmul(cps1, cwt, olnT, wprojT, C, C, consume_proj, "pw")

                    for t in range(RT):
                        h2 = cw.tile([128, C], BF16, name="h2", tag="h2")
                        ln_apply(h2, x1_my[t], C, "a2ln", apply_eng="act")
                        transpose_to(cps, h2T[:, :, t * 128:(t + 1) * 128], h2,
                                     KIN, (lambda out, in_:
                                           nc.scalar.copy(out=out, in_=in_)))

                    # q2 build
                    for mh in range(2):
                        wts = []
                        for K in range(KIN):
                            wt = cwt.tile([128, 512], BF16, name=f"wq2{K}",
                                          tag=f"wq2{K}", bufs=1)
                            nc.sync.dma_start(
                                out=wt, in_=wq2T[K * 128:(K + 1) * 128,
                                                 mh * 512:(mh + 1) * 512])
                            wts.append(wt)
                        for mm in range(4):
                            m = mh * 4 + mm
                            ps = cps1.tile([128, R], F32, name="q2ps", tag="q2ps")
                            for K in range(KIN):
                                nc.tensor.matmul(
                                    ps, wts[K][:, mm * 128:(mm + 1) * 128],
                                    h2T[:, K, :], start=(K == 0),
                                    stop=(K == KIN - 1))
                            nc.scalar.mul(out=q2T[:, m, :], in_=ps, mul=SCALE)

                # cross k2/v2 builds: emitted after C so C's critical
                # DMAs take queue priority; matmuls still fill B/C PE gaps
                if True:
                    # k2 build: n-chunk outer (y chunks loaded once), resident
                    # k-half weights
                    k2wts = []
                    for K in range(KIN):
                        wt = bkv2w.tile([128, C], BF16, name=f"wk2{K}",
                                        tag=f"wk2{K}", bufs=1)
                        nc.sync.dma_start(out=wt,
                                          in_=wkv2T[K * 128:(K + 1) * 128, 0:C])
                        k2wts.append(wt)
                    for n in range(N // 512):
                        ycs = []
                        for K in range(KIN):
                            yc = bkv2w.tile([128, 512], BF16, name=f"yc{K}",
                                            tag=f"yc{K}", bufs=1)
                            nc.sync.dma_start(
                                out=yc, in_=yTb[K * 128:(K + 1) * 128,
                                                n * 512:(n + 1) * 512])
                            ycs.append(yc)
                        for m in range(KIN):
                            ps = bkv2ps.tile([128, 512], F32, name="k2ps",
                                             tag="kv2ps")
                            for K in range(KIN):
                                nc.tensor.matmul(
                                    ps, k2wts[K][:, m * 128:(m + 1) * 128],
                                    ycs[K], start=(K == 0), stop=(K == KIN - 1))
                            nc.vector.tensor_copy(
                                out=k2T[m][:, n * 512:(n + 1) * 512], in_=ps)

                    # v2 build: y blocks streamed per key tile
                    for half in range(2):
                        v2wts = []
                        for K in range(KIN):
                            wt = bkv2w.tile([128, 512], BF16, name=f"wv2{K}",
                                            tag=f"wv2{K}", bufs=1)
                            nc.sync.dma_start(
                                out=wt, in_=wkv2T[K * 128:(K + 1) * 128,
                                                  C + half * 512:
                                                  C + (half + 1) * 512])
                            v2wts.append(wt)
                        for t in range(KB):
                            yb = bkv2w.tile([128, KIN, 128], BF16, name="yb",
                                            tag="yb", bufs=2)
                            nc.sync.dma_start(
                                out=yb,
                                in_=yBlk[t * 128:(t + 1) * 128, :]
                                .rearrange("p (K k) -> p K k", k=128))
                            ps = bkv2ps.tile([128, 512], F32, name="v2ps",
                                             tag="kv2ps")
                            for K in range(KIN):
                                nc.tensor.matmul(ps, yb[:, K, :], v2wts[K],
                                                 start=(K == 0),
                                                 stop=(K == KIN - 1))
                            dst = v2_p[t // 2].rearrange("p a (h c) -> p a h c",
                                                         c=65)
                            nc.vector.tensor_copy(
                                out=dst[:, t % 2, half * 8:(half + 1) * 8, 0:64],
                                in_=ps.rearrange("p (h c) -> p h c", c=64))
                    for jp in range(NP):
                        dst = v2_p[jp].rearrange("p a (h c) -> p a h c", c=65)
                        nc.vector.memset(dst[:, :, :, 64:65], 1.0)


                _cm_bkv2w.__exit__(None, None, None)
                _cm_bkv2ps.__exit__(None, None, None)

                # ---- stage D: cross attention ------------------------------
                with tc.tile_pool(name="datt", bufs=3) as datt, \
                     tc.tile_pool(name="dlg", bufs=2, space="PSUM") as dlg, \
                     tc.tile_pool(name="dops", bufs=1, space="PSUM") as dops, \
                     tc.tile_pool(name="dtp", bufs=1, space="PSUM") as dtp:
                    attention(q2T, k2T, v2_p, consume_row_head(o2_sb, dtp),
                              causal=False, lgp=dlg, opsp=dops, ptp=datt,
                              msk=None)

                _cm_kv2.__exit__(None, None, None)  # cross K/V dead

                # ---- stage E: proj2 + residual; MLP ------------------------
                with tc.tile_pool(name="ew", bufs=2) as ew, \
                     tc.tile_pool(name="eh4", bufs=1) as eh4, \
                     tc.tile_pool(name="ewt", bufs=4) as ewt, \
                     tc.tile_pool(name="ewf", bufs=16) as ewf, \
                     tc.tile_pool(name="eps2", bufs=2, space="PSUM") as eps2, \
                     tc.tile_pool(name="eps1", bufs=1, space="PSUM") as eps1:
                    x2_my = [eh4.tile([128, C], F32, name=f"x2my{t}",
                                      tag=f"x2my{t}") for t in range(RT)]
                    o2T = eh4.tile([128, KIN, R], BF16, name="o2T", tag="o2T")
                    for t in range(RT):
                        transpose_to(eps2, o2T[:, :, t * 128:(t + 1) * 128],
                                     o2_sb[t], KIN,
                                     (lambda out, in_:
                                      nc.vector.tensor_copy(out=out, in_=in_)))

                    def consume_proj2(rt, nch, ps):
                        sl = slice(nch * 512, (nch + 1) * 512)
                        if not skip_bias:
                            nc.vector.tensor_tensor(
                                out=ps, in0=ps,
                                in1=bias_tiles["proj2b"][:, sl], op=ALU.add)
                        nc.vector.tensor_tensor(out=x2_my[rt][:, sl], in0=ps,
                                                in1=x1_my[rt][:, sl], op=ALU.add)

                    rows_matmul(eps1, ewt, o2T, wproj2T, C, C, consume_proj2,
                                "p2w")

                    # MLP
                    h3T = eh4.tile([128, KIN, R], BF16, name="h3T", tag="h3T")
                    h4 = [eh4.tile([128, HID], BF16, name=f"h4_{t}",
                                   tag=f"h4_{t}") for t in range(RT)]
                    h5T = eh4.tile([128, HID // 128, R], BF16, name="h5T",
                                   tag="h5T")
                    for t in range(RT):
                        h3 = ew.tile([128, C], BF16, name="h3", tag="h3")
                        ln_apply(h3, x2_my[t], C, "mln1", apply_eng="act")
                        transpose_to(eps2, h3T[:, :, t * 128:(t + 1) * 128], h3,
                                     KIN, (lambda out, in_:
                                           nc.vector.tensor_copy(out=out,
                                                                 in_=in_)))

                    mst = [eh4.tile([128, HID // 512, 6], F32,
                                    name=f"mst{t}", tag=f"mst{t}")
                           for t in range(RT)]

                    def consume_fc1(rt, nch, ps):
                        sl = slice(nch * 512, (nch + 1) * 512)
                        if not skip_bias:
                            nc.vector.tensor_tensor(
                                out=ps, in0=ps,
                                in1=bias_tiles["fc1b"][:, sl], op=ALU.add)
                        nc.scalar.activation(out=h4[rt][:, sl], in_=ps,
                                             func=AF.Gelu)
                        nc.vector.bn_stats(out=mst[rt][:, nch, :],
                                           in_=h4[rt][:, sl])

                    rows_matmul(eps1, ewf, h3T, wfc1T, HID, C, consume_fc1,
                                "f1w")

                    for t in range(RT):
                        h5 = ew.tile([128, HID], BF16, name="h5", tag="h5",
                                     bufs=1)
                        if not skip_gb:
                            ln_apply(h5, h4[t], HID, "mln2", apply_eng="dve",
                                     st_in=mst[t])
                        else:
                            mv = stats.tile([128, 2], F32, name="bnmv",
                                            tag="bnmv")
                            nc.vector.bn_aggr(out=mv, in_=mst[t])
                            sd = stats.tile([128, 1], F32, name="bnsd",
                                            tag="bnsd")
                            nc.scalar.activation(out=sd, in_=mv[:, 1:2],
                                                 func=AF.Sqrt, bias=eps_t)
                            ri = stats.tile([128, 1], F32, name="h5ri",
                                            tag="h5ri")
                            nc.vector.reciprocal(out=ri, in_=sd)
                            nm = stats.tile([128, 1], F32, name="h5nm",
                                            tag="h5nm")
                            nc.vector.tensor_scalar(out=nm, in0=mv[:, 0:1],
                                                    scalar1=ri, scalar2=-1.0,
                                                    op0=ALU.mult, op1=ALU.mult)
                        for g in range(0, HID // 128, 8):
                            sl = slice(g * 128, (g + 8) * 128)
                            if skip_gb:
                                if (g // 8) % 2 == 0:
                                    nc.vector.tensor_scalar(
                                        out=h5[:, sl], in0=h4[t][:, sl],
                                        scalar1=ri, scalar2=nm, op0=ALU.mult,
                                        op1=ALU.add)
                                else:
                                    nc.scalar.activation(
                                        out=h5[:, sl], in_=h4[t][:, sl],
                                        func=AF.Identity, bias=nm, scale=ri)
                            transpose_to(
                                eps2,
                                h5T[:, g:g + 8, t * 128:(t + 1) * 128],
                                h5[:, sl], 8,
                                (lambda out, in_:
                                 nc.vector.tensor_copy(out=out, in_=in_))
                                if (g // 8) % 2 == 0 else
                                (lambda out, in_: nc.scalar.copy(out=out,
                                                                 in_=in_)))

                    def consume_fc2(rt, nch, ps):
                        sl = slice(nch * 512, (nch + 1) * 512)
                        if not skip_bias:
                            nc.vector.tensor_tensor(
                                out=ps, in0=ps,
                                in1=bias_tiles["fc2b"][:, sl], op=ALU.add)
                        x3 = ew.tile([128, 512], F32, name="x3", tag="x3")
                        nc.vector.tensor_tensor(out=x3, in0=ps,
                                                in1=x2_my[rt][:, sl], op=ALU.add)
                        nc.sync.dma_start(
                            out=out_my[rt * 128:(rt + 1) * 128, sl], in_=x3)

                    rows_matmul(eps1, ewf, h5T, wfc2T, C, HID, consume_fc2,
                                "f2w")

                _cm_x1p.__exit__(None, None, None)

    nc.compile()
    return nc


# ---------------------------------------------------------------------------
# host side
# ---------------------------------------------------------------------------

def _host_prep(inputs):
    f32 = np.float32
    x = np.asarray(inputs["x"], f32)
    y = np.asarray(inputs["y"], f32)
    mask = np.asarray(inputs["mask"])[0, 0]  # [N, N] bool

    causal_ref = np.triu(np.ones((N, N), bool), k=1)
    if np.array_equal(mask, causal_ref):
        mode = "causal"
    elif not mask.any():
        mode = "none"
    else:
        mode = "dense"

    gbs = [("a1_ln1_g", "a1_ln1_b"), ("a1_ln2_g", "a1_ln2_b"),
           ("a2_ln_g", "a2_ln_b"), ("m_ln1_g", "m_ln1_b"), ("m_ln2_g", "m_ln2_b")]
    skip_gb = all(
        np.all(np.asarray(inputs[g]) == 1.0) and np.all(np.asarray(inputs[b]) == 0.0)
        for g, b in gbs)
    skip_bias = all(np.all(np.asarray(inputs[k]) == 0.0)
                    for k in ("a1_proj_b", "a2_proj_b", "m_fc1_b", "m_fc2_b"))

    wTb = lambda k: np.ascontiguousarray(
        np.asarray(inputs[k], f32).T).astype(BF16NP)
    shared = {
        "wqkT": wTb("a1_qk_w"),
        "wvT": wTb("a1_v_w"),
        "wq2T": wTb("a2_q_w"),
        "wkv2T": wTb("a2_kv_w"),
        "wprojT": wTb("a1_proj_w"),
        "wproj2T": wTb("a2_proj_w"),
        "wfc1T": wTb("m_fc1_w"),
        "wfc2T": wTb("m_fc2_w"),
    }
    if not skip_gb:
        keymap = {"ln1": ("a1_ln1_g", "a1_ln1_b"), "aln2": ("a1_ln2_g", "a1_ln2_b"),
                  "a2ln": ("a2_ln_g", "a2_ln_b"), "mln1": ("m_ln1_g", "m_ln1_b"),
                  "mln2": ("m_ln2_g", "m_ln2_b")}
        for k, (gk, bk) in keymap.items():
            shared["g_" + k] = np.asarray(inputs[gk], f32)
            shared["b_" + k] = np.asarray(inputs[bk], f32)
    if not skip_bias:
        shared["projb"] = np.asarray(inputs["a1_proj_b"], f32)
        shared["proj2b"] = np.asarray(inputs["a2_proj_b"], f32)
        shared["fc1b"] = np.asarray(inputs["m_fc1_b"], f32)
        shared["fc2b"] = np.asarray(inputs["m_fc2_b"], f32)

    in_maps = []
    for core in range(G * CPG):
        g, c = core // CPG, core % CPG
        m = dict(shared)
        m["x_my"] = np.ascontiguousarray(x[g, c::CPG])
        m["x_full"] = np.ascontiguousarray(x[g])
        yT_g = np.ascontiguousarray(y[g].T).astype(BF16NP)
        m["yTb"] = yT_g
        m["yBlk"] = np.ascontiguousarray(
            yT_g.reshape(KIN, 128, KB, 128).transpose(2, 1, 0, 3)
            .reshape(KB * 128, KIN * 128))
        if mode == "causal":
            # pair mask [128 keys, 2 blocks, 64 range-rows]: block b allows
            # key kk iff kk <= c + 4*d - 128*b  (d = row offset in range)
            kk = np.arange(128)[:, None, None]
            bb = np.arange(2)[None, :, None]
            dd = np.arange(64)[None, None, :]
            pm = (kk <= c + 4 * dd - 128 * bb).astype(f32)
            m["pmask"] = pm.reshape(128, 128).astype(BF16NP)
        if mode == "dense":
            sub = (~mask[c::CPG, :]).astype(f32)  # [R, N] rows x keys
            dm = np.empty((NP * 128, 2 * R), f32)
            for jp in range(NP):
                for b in range(2):
                    blk = sub[:, (2 * jp + b) * 128:(2 * jp + b + 1) * 128].T
                    dm[jp * 128:(jp + 1) * 128, b * R:(b + 1) * R] = blk
            m["dmask"] = dm.astype(BF16NP)
        in_maps.append(m)
    return mode, skip_gb, skip_bias, in_maps


def _assemble(results, dtype):
    out = np.empty((B, N, C), np.float32)
    for core in range(G * CPG):
        g, c = core // CPG, core % CPG
        out[g, c::CPG] = results[core]["out_my"]
    return out.astype(dtype, copy=False)


def get_program(inputs):
    """Build (or fetch cached) program + per-core input maps for these inputs."""
    mode, skip_gb, skip_bias, in_maps = _host_prep(inputs)
    key = (mode, skip_gb, skip_bias)
    if key not in _CACHE:
        _CACHE[key] = _build(mode, skip_gb, skip_bias)
    return _CACHE[key], in_maps


def kernel(**inputs):
    from concourse import bass_utils

    nc, in_maps = get_program(inputs)
    res = bass_utils.run_bass_kernel_spmd(nc, in_maps, core_ids=list(range(8)))
    return _assemble(res.results, np.asarray(inputs["x"]).dtype)
